# revision 30
# baseline (speedup 1.0000x reference)
"""Distributed multi-head attention kernel for 8 TRN2 NeuronCores.

Problem: B=2, N=2048, C=1024, H=16 heads, D=64.
  out = softmax((q@Wq)(k@Wk)^T / sqrt(D)) @ (v@Wv) @ Wo   (per head, biases are zero)

Sharding: data-parallel over batch x tensor-parallel over heads.  Core c owns
batch b=c//4 and heads [4*(c%4), 4*(c%4)+4).  Each core projects Q/K/V for its
4 heads only (full token range), runs attention locally, and computes its
row-shard of the output projection; the host sums the 4 partial outputs per
batch (free in HW time).  This removes the 4x-redundant full-batch K/V
projections of the old sequence-parallel layout: per-core PE work drops from
~590K to ~393K moving-rows.

Engine budget per core (measured): ScalarE exp of 16.8M scores (~150us) is the
attention floor; PE ~393K moving-rows (~170us).  DVE work is kept minimal by
design: DVE activity power-throttles the PE to a 0.5 utilization limit
(measured via throttle_activity_1 in the NTFF profile), so exp runs 100% on
ScalarE and only small copies/normalize ops use DVE during attention.

Emission order (single in-order queue per engine) weaves phases so ScalarE
never starves and the PE fills exp-bound gaps:
  Qproj(heads 0,1) -> Kproj(heads 0,1) -> unit0 scores -> Vproj ->
  unit1+PV(0) -> unit2+PV(1) -> Qproj(2,3) -> Kproj(2,3) woven into
  unit3/unit4 ... unit7+PV(6) -> PV(7) -> out-projection.
Each attention unit = (head, 1024-query slab): 16 key-chunk score matmuls
[128 keys x 2x512 q] + 16 ScalarE exp slices; PV accumulates A^T[65,1024]
(V' column 64 = ones => row 64 = softmax denominator), then normalize -> aT.
"""

import sys

sys.path.insert(0, "/opt/trn_rl_repo")

from contextlib import ExitStack

import numpy as np
import ml_dtypes

import concourse.bass as bass
import concourse.bacc as bacc
import concourse.mybir as mybir
import concourse.tile as tile
from concourse.bass_utils import run_bass_kernel_spmd

BF16 = mybir.dt.bfloat16
F32 = mybir.dt.float32
I16 = mybir.dt.int16
Exp = mybir.ActivationFunctionType.Exp
Mult = mybir.AluOpType.mult
Add = mybir.AluOpType.add

B, N, C = 2, 2048, 1024
H, D = 16, 64
HL = 4               # heads per core
CL = HL * D          # local channels = 256
NKC = N // 128       # 16 key chunks
QP = 1024            # query slab in attention units
SCALE = 0.125
SCALAR_KC = 13       # kc slices per unit on ScalarE; rest via DVE Schraudolph

# Single-op DVE exp: bf16 bits = int16(s*A16 + B16); the bf16 mantissa is a
# linear approx of 2^frac (Schraudolph), rel err ~1.8% rms on 4/16 of the
# softmax mass => ~0.9% on the output, inside the 2e-2 budget.
A16 = float(SCALE * 128.0 / np.log(2.0))
B16 = float(16256.0 - 7.25)

_CACHE = {}


def build_nc():
    nc = bacc.Bacc("TRN2", target_bir_lowering=False, debug=False, num_devices=8)

    # host-packed layouts: x* are [p, (cc, n)], w* are [p, (cc, m)], wo [p, (p2, m)]
    xq = nc.declare_dram_parameter("xq", [128, 8 * N], BF16, isOutput=False)
    xk = nc.declare_dram_parameter("xk", [128, 8 * N], BF16, isOutput=False)
    xv = nc.declare_dram_parameter("xv", [128, 8 * N], BF16, isOutput=False)
    wq = nc.declare_dram_parameter("wq", [128, 8 * CL], BF16, isOutput=False)
    wk = nc.declare_dram_parameter("wk", [128, 8 * CL], BF16, isOutput=False)
    wv = nc.declare_dram_parameter("wv", [128, 8 * CL], BF16, isOutput=False)
    wo = nc.declare_dram_parameter("wo", [128, 2 * C], BF16, isOutput=False)
    outT = nc.declare_dram_parameter("outT", [C, N], BF16, isOutput=True)

    with tile.TileContext(nc) as tc, ExitStack() as top:
        # ---------------- resident SBUF ----------------
        res = top.enter_context(tc.tile_pool(name="res", bufs=1))
        qT = [res.tile([128, N], BF16, tag=f"qT{p}", name=f"qT{p}")
              for p in range(2)]
        kT = [res.tile([128, N], BF16, tag=f"kT{p}", name=f"kT{p}")
              for p in range(2)]
        vpA = res.tile([128, NKC * HL * 65], BF16, tag="vpA", name="vpA")
        aT = [res.tile([128, N], BF16, tag=f"aT{p}", name=f"aT{p}")
              for p in range(2)]

        inp_v = ExitStack()
        attn = ExitStack()
        spool = attn.enter_context(
            tc.tile_pool(name="spool", bufs=2, space="PSUM"))
        apool = attn.enter_context(
            tc.tile_pool(name="apool", bufs=2, space="PSUM"))
        Ppool = attn.enter_context(tc.tile_pool(name="Ppool", bufs=2))
        pools = {}
        ipq = attn.enter_context(tc.tile_pool(name="ipq", bufs=1))
        ipk = attn.enter_context(tc.tile_pool(name="ipk", bufs=1))
        ipv = inp_v.enter_context(tc.tile_pool(name="ipv", bufs=1))
        xq_sb = ipq.tile([128, 8 * N], BF16, tag="xq", name="xq_sb")
        xk_sb = ipk.tile([128, 8 * N], BF16, tag="xk", name="xk_sb")
        xv_sb = ipv.tile([128, 8 * N], BF16, tag="xv", name="xv_sb")
        wq_sb = ipq.tile([128, 8 * CL], BF16, tag="wq", name="wq_sb")
        wk_sb = ipk.tile([128, 8 * CL], BF16, tag="wk", name="wk_sb")
        wv_sb = ipv.tile([128, 8 * CL], BF16, tag="wv", name="wv_sb")

        # DMAs split in halves so the first projection matmuls start early
        nc.sync.dma_start(out=wq_sb[:], in_=wq[:, :])
        for hf in range(2):
            sl = slice(4 * N * hf, 4 * N * (hf + 1))
            nc.sync.dma_start(out=xq_sb[:, sl], in_=xq[:, sl])
        nc.sync.dma_start(out=wk_sb[:], in_=wk[:, :])
        for hf in range(2):
            sl = slice(4 * N * hf, 4 * N * (hf + 1))
            nc.sync.dma_start(out=xk_sb[:, sl], in_=xk[:, sl])
        nc.sync.dma_start(out=wv_sb[:], in_=wv[:, :])
        for hf in range(2):
            sl = slice(4 * N * hf, 4 * N * (hf + 1))
            nc.sync.dma_start(out=xv_sb[:, sl], in_=xv[:, sl])

        xq3 = xq_sb[:].rearrange("p (cc n) -> p cc n", cc=8)
        xk3 = xk_sb[:].rearrange("p (cc n) -> p cc n", cc=8)
        xv3 = xv_sb[:].rearrange("p (cc n) -> p cc n", cc=8)
        wq3 = wq_sb[:].rearrange("p (cc m) -> p cc m", cc=8)
        wk3 = wk_sb[:].rearrange("p (cc m) -> p cc m", cc=8)
        wv3 = wv_sb[:].rearrange("p (cc m) -> p cc m", cc=8)
        vp4 = vpA[:].rearrange("p (kc h x) -> p kc h x", kc=NKC, x=65)
        nc.vector.memset(vp4[:, :, :, 64:65], 1.0)

        def emit_qk_proj(w3, x3, dst, mt, copy_engine, qcps=(0, 1)):
            # channels [128*mt, 128*mt+128) x all 2048 tokens; a spool tile
            # holds a query-chunk pair in its two PSUM banks
            for qcp in qcps:
                ps = spool.tile([128, QP], F32, tag="st",
                                name=f"pj{mt}_{qcp}")
                for cc in range(8):
                    for i in range(2):
                        nc.tensor.matmul(
                            ps[:, 512 * i:512 * (i + 1)],
                            w3[:, cc, 128 * mt:128 * (mt + 1)],
                            x3[:, cc, 512 * (2 * qcp + i):
                               512 * (2 * qcp + i + 1)],
                            start=(cc == 0), stop=(cc == 7))
                for i in range(2):
                    qc = 2 * qcp + i
                    if copy_engine == "scalar":
                        nc.scalar.copy(dst[mt][:, 512 * qc:512 * (qc + 1)],
                                       ps[:, 512 * i:512 * (i + 1)])
                    else:
                        nc.vector.tensor_copy(
                            dst[mt][:, 512 * qc:512 * (qc + 1)],
                            ps[:, 512 * i:512 * (i + 1)])

        def emit_v_proj():
            # V'[128 keys, 4 heads x 64] per key tile; stationary = xv chunk
            for kt in range(NKC):
                psv = spool.tile([128, QP], F32, tag="st", name=f"v{kt}")
                for cc in range(8):
                    nc.tensor.matmul(
                        psv[:, 0:CL],
                        xv3[:, cc, 128 * kt:128 * (kt + 1)],
                        wv3[:, cc, :],
                        start=(cc == 0), stop=(cc == 7))
                nc.vector.tensor_copy(
                    vp4[:, kt, :, 0:64],
                    psv[:, 0:CL].rearrange("p (h d) -> p h d", d=D))

        def emit_pv_mm(pr, kc):
            h, up, P3 = pr[0], pr[1], pr[2]
            if pr[3] is None:
                pr[3] = apool.tile([128, QP], F32, tag="A", name=f"A{h}_{up}")
            At = pr[3]
            for hf in range(2):
                nc.tensor.matmul(
                    At[0:65, 512 * hf:512 * (hf + 1)],
                    vp4[:, kc, h, :],
                    P3[:, kc, 512 * hf:512 * (hf + 1)],
                    start=(kc == 0), stop=(kc == NKC - 1))

        def emit_norm(pr):
            # den copy and row-broadcast ride the DMA engines (no PE throttle)
            h, up, P3, At = pr
            assert At is not None
            dpool = pools["dpool"]
            den = dpool.tile([1, QP], F32, tag="den", name=f"d{h}_{up}")
            nc.vector.tensor_copy(den[:], At[64:65, :])
            dinv = dpool.tile([1, QP], F32, tag="dinv", name=f"r{h}_{up}")
            nc.vector.reciprocal_approx_fast(dinv[:], den[:])
            dib = dpool.tile([64, QP], F32, tag="dib", name=f"b{h}_{up}")
            nc.gpsimd.partition_broadcast(dib[:], dinv[:])
            nc.vector.tensor_mul(
                aT[h // 2][64 * (h % 2):64 * (h % 2) + 64,
                           QP * up:QP * (up + 1)],
                At[0:64, :], dib[:])

        # ---- pipelined emission ----
        units = [(h, up) for h in range(HL) for up in range(2)]
        def after_v():
            emit_qk_proj(wq3, xq3, qT, 0, "vector", qcps=(1,))
            emit_v_proj()
            inp_v.close()
            pools["dpool"] = attn.enter_context(
                tc.tile_pool(name="dpool", bufs=1))

        def after_q():
            emit_qk_proj(wq3, xq3, qT, 1, "vector")

        def after_k():
            emit_qk_proj(wk3, xk3, kT, 1, "vector")

        extra = {0: [after_v], 1: [after_q], 2: [after_k]}

        emit_qk_proj(wq3, xq3, qT, 0, "scalar", qcps=(0,))
        emit_qk_proj(wk3, xk3, kT, 0, "vector", qcps=(0,))

        prev = None
        last_u = len(units) - 1
        for u, (h, up) in enumerate(units):
            p, r = h // 2, h % 2
            Pt = Ppool.tile([128, NKC * QP], BF16, tag="P", name=f"P{u}")
            P3 = Pt[:].rearrange("p (kc q) -> p kc q", kc=NKC)
            cur = [h, up, P3, None]
            for kc in range(NKC):
                st = spool.tile([128, QP], F32, tag="st", name=f"s{u}_{kc}")
                for hf in range(2):
                    nc.tensor.matmul(
                        st[:, 512 * hf:512 * (hf + 1)],
                        kT[p][64 * r:64 * (r + 1), 128 * kc:128 * (kc + 1)],
                        qT[p][64 * r:64 * (r + 1),
                              QP * up + 512 * hf:QP * up + 512 * (hf + 1)],
                        start=True, stop=True)
                if kc < SCALAR_KC:
                    nc.scalar.activation(P3[:, kc, :], st[:], Exp,
                                         scale=float(SCALE))
                else:
                    nc.vector.tensor_scalar(
                        P3[:, kc, :].bitcast(I16), st[:], A16, B16, Mult, Add)
                if prev is not None:
                    emit_pv_mm(prev, kc)
                if u == last_u and kc > 0:
                    emit_pv_mm(cur, kc - 1)   # tight PV in the last unit
                if u == 0 and kc == 7:
                    emit_qk_proj(wk3, xk3, kT, 0, "vector", qcps=(1,))
            if prev is not None:
                emit_norm(prev)
            for fn in extra.get(u, []):
                fn()
            prev = cur
        emit_pv_mm(prev, NKC - 1)
        emit_norm(prev)
        attn.close()

        # ---------------- output projection ----------------
        with ExitStack() as ph:
            opool = ph.enter_context(
                tc.tile_pool(name="opool", bufs=2, space="PSUM"))
            epool = ph.enter_context(tc.tile_pool(name="epool", bufs=3))
            wo_sb = epool.tile([128, 2 * C], BF16, tag="wo", name="wo_sb")
            nc.sync.dma_start(out=wo_sb[:], in_=wo[:, :])
            # qp-major: the qp=0 pieces depend only on norms finished a
            # unit earlier, so the PE isn't queue-blocked on the last
            # unit's normalize chain
            for qp in range(2):
                for mt in range(8):
                    ps = opool.tile([128, QP], F32, tag="po",
                                    name=f"o{mt}_{qp}")
                    for p in range(2):
                        for hf in range(2):
                            nc.tensor.matmul(
                                ps[:, 512 * hf:512 * (hf + 1)],
                                wo_sb[:, C * p + 128 * mt:
                                      C * p + 128 * (mt + 1)],
                                aT[p][:, QP * qp + 512 * hf:
                                      QP * qp + 512 * (hf + 1)],
                                start=(p == 0), stop=(p == 1))
                    ev = epool.tile([128, QP], BF16, tag="ev",
                                    name=f"e{mt}_{qp}")
                    if mt % 2 == 0:
                        nc.scalar.copy(ev[:], ps[:])
                    else:
                        nc.vector.tensor_copy(ev[:], ps[:])
                    nc.sync.dma_start(
                        out=outT[128 * mt:128 * (mt + 1),
                                 QP * qp:QP * (qp + 1)],
                        in_=ev[:])

    nc.compile()
    return nc


def _get_nc():
    if "nc" not in _CACHE:
        _CACHE["nc"] = build_nc()
    return _CACHE["nc"]


def _pack_x(xT, bf):
    # [C, N] -> [128, (cc, N)]
    return np.ascontiguousarray(
        xT.reshape(8, 128, N).transpose(1, 0, 2).reshape(128, 8 * N)).astype(bf)


def _pack_w(w_loc, bf):
    # [C, CL] -> [128, (cc, CL)]
    return np.ascontiguousarray(
        w_loc.reshape(8, 128, CL).transpose(1, 0, 2).reshape(128, 8 * CL)
    ).astype(bf)


def _make_in_maps(q, k, v, Wq, Wk, Wv, Wo):
    bf = ml_dtypes.bfloat16
    q, k, v = np.asarray(q), np.asarray(k), np.asarray(v)
    Wq, Wk, Wv, Wo = (np.asarray(a) for a in (Wq, Wk, Wv, Wo))
    xq_b = [_pack_x(q[b].T, bf) for b in range(B)]
    xk_b = [_pack_x(k[b].T, bf) for b in range(B)]
    xv_b = [_pack_x(v[b].T, bf) for b in range(B)]
    in_maps = []
    for c in range(8):
        b, hg = c // 4, c % 4
        hsl = slice(CL * hg, CL * (hg + 1))
        wo_loc = np.ascontiguousarray(
            Wo[hsl, :].reshape(2, 128, C).transpose(1, 0, 2).reshape(128, 2 * C)
        ).astype(bf)
        in_maps.append({
            "xq": xq_b[b], "xk": xk_b[b], "xv": xv_b[b],
            "wq": _pack_w(Wq[:, hsl], bf),
            "wk": _pack_w(Wk[:, hsl], bf),
            "wv": _pack_w(Wv[:, hsl], bf),
            "wo": wo_loc,
        })
    return in_maps


def _run(inputs, trace=False, **kw):
    nc = _get_nc()
    in_maps = _make_in_maps(inputs["q"], inputs["k"], inputs["v"],
                            inputs["Wq"], inputs["Wk"], inputs["Wv"],
                            inputs["Wo"])
    res = None
    for attempt in range(3):
        try:
            res = run_bass_kernel_spmd(nc, in_maps, core_ids=list(range(8)),
                                       trace=trace, **kw)
            break
        except Exception:
            if attempt == 2:
                raise
            import time
            time.sleep(2.0)
    out = np.zeros((B, N, C), np.float32)
    for c in range(8):
        b = c // 4
        out[b] += res.results[c]["outT"].T.astype(np.float32)
    return out, res


def kernel(**inputs) -> np.ndarray:
    out, _ = _run(inputs, trace=False)
    return out


# revision 31
# speedup vs baseline: 1.0698x; 1.0698x over previous
"""Distributed multi-head attention kernel for 8 TRN2 NeuronCores.

Problem: B=2, N=2048, C=1024, H=16 heads, D=64.
  out = softmax((q@Wq)(k@Wk)^T / sqrt(D)) @ (v@Wv) @ Wo   (per head, biases are zero)

Sharding: data-parallel over batch x tensor-parallel over heads.  Core c owns
batch b=c//4 and heads [4*(c%4), 4*(c%4)+4).  Each core projects Q/K/V for its
4 heads only (full token range), runs attention locally, and computes its
row-shard of the output projection; the host sums the 4 partial outputs per
batch (free in HW time).  This removes the 4x-redundant full-batch K/V
projections of the old sequence-parallel layout: per-core PE work drops from
~590K to ~393K moving-rows.

Engine budget per core (measured): ScalarE exp of 16.8M scores (~150us) is the
attention floor; PE ~393K moving-rows (~170us).  DVE work is kept minimal by
design: DVE activity power-throttles the PE to a 0.5 utilization limit
(measured via throttle_activity_1 in the NTFF profile), so exp runs 100% on
ScalarE and only small copies/normalize ops use DVE during attention.

Emission order (single in-order queue per engine) weaves phases so ScalarE
never starves and the PE fills exp-bound gaps:
  Qproj(heads 0,1) -> Kproj(heads 0,1) -> unit0 scores -> Vproj ->
  unit1+PV(0) -> unit2+PV(1) -> Qproj(2,3) -> Kproj(2,3) woven into
  unit3/unit4 ... unit7+PV(6) -> PV(7) -> out-projection.
Each attention unit = (head, 1024-query slab): 16 key-chunk score matmuls
[128 keys x 2x512 q] + 16 ScalarE exp slices; PV accumulates A^T[65,1024]
(V' column 64 = ones => row 64 = softmax denominator), then normalize -> aT.
"""

import sys

sys.path.insert(0, "/opt/trn_rl_repo")

from contextlib import ExitStack

import numpy as np
import ml_dtypes

import concourse.bass as bass
import concourse.bacc as bacc
import concourse.mybir as mybir
import concourse.tile as tile
from concourse.bass_utils import run_bass_kernel_spmd

BF16 = mybir.dt.bfloat16
F32 = mybir.dt.float32
I16 = mybir.dt.int16
Exp = mybir.ActivationFunctionType.Exp
Mult = mybir.AluOpType.mult
Add = mybir.AluOpType.add

B, N, C = 2, 2048, 1024
H, D = 16, 64
HL = 4               # heads per core
CL = HL * D          # local channels = 256
NKC = N // 128       # 16 key chunks
QP = 1024            # query slab in attention units
SCALE = 0.125
SCALAR_KC = 13       # kc slices per unit on ScalarE; rest via DVE Schraudolph

# Single-op DVE exp: bf16 bits = int16(s*A16 + B16); the bf16 mantissa is a
# linear approx of 2^frac (Schraudolph), rel err ~1.8% rms on 4/16 of the
# softmax mass => ~0.9% on the output, inside the 2e-2 budget.
A16 = float(SCALE * 128.0 / np.log(2.0))
B16 = float(16256.0 - 7.25)

_CACHE = {}


def build_nc():
    nc = bacc.Bacc("TRN2", target_bir_lowering=False, debug=False, num_devices=8)

    # host-packed layouts: x* are [p, (cc, n)], w* are [p, (cc, m)], wo [p, (p2, m)]
    xq = nc.declare_dram_parameter("xq", [128, 8 * N], BF16, isOutput=False)
    xk = nc.declare_dram_parameter("xk", [128, 8 * N], BF16, isOutput=False)
    xv = nc.declare_dram_parameter("xv", [128, 8 * N], BF16, isOutput=False)
    wq = nc.declare_dram_parameter("wq", [128, 8 * CL], BF16, isOutput=False)
    wk = nc.declare_dram_parameter("wk", [128, 8 * CL], BF16, isOutput=False)
    wv = nc.declare_dram_parameter("wv", [128, 8 * CL], BF16, isOutput=False)
    wo = nc.declare_dram_parameter("wo", [128, 2 * C], BF16, isOutput=False)
    outT = nc.declare_dram_parameter("outT", [C, N], BF16, isOutput=True)

    with tile.TileContext(nc) as tc, ExitStack() as top:
        # ---------------- resident SBUF ----------------
        res = top.enter_context(tc.tile_pool(name="res", bufs=1))
        qT = [res.tile([128, N], BF16, tag=f"qT{p}", name=f"qT{p}")
              for p in range(2)]
        kT = [res.tile([128, N], BF16, tag=f"kT{p}", name=f"kT{p}")
              for p in range(2)]
        vpA = res.tile([128, NKC * HL * 65], BF16, tag="vpA", name="vpA")
        aT = [res.tile([128, N], BF16, tag=f"aT{p}", name=f"aT{p}")
              for p in range(2)]

        inp_v = ExitStack()
        attn = ExitStack()
        spool = attn.enter_context(
            tc.tile_pool(name="spool", bufs=2, space="PSUM"))
        apool = attn.enter_context(
            tc.tile_pool(name="apool", bufs=2, space="PSUM"))
        Ppool = attn.enter_context(tc.tile_pool(name="Ppool", bufs=2))
        pools = {}
        ipq = attn.enter_context(tc.tile_pool(name="ipq", bufs=1))
        ipk = attn.enter_context(tc.tile_pool(name="ipk", bufs=1))
        ipv = inp_v.enter_context(tc.tile_pool(name="ipv", bufs=1))
        xq_sb = ipq.tile([128, 8 * N], BF16, tag="xq", name="xq_sb")
        xk_sb = ipk.tile([128, 8 * N], BF16, tag="xk", name="xk_sb")
        xv_sb = ipv.tile([128, 8 * N], BF16, tag="xv", name="xv_sb")
        wq_sb = ipq.tile([128, 8 * CL], BF16, tag="wq", name="wq_sb")
        wk_sb = ipk.tile([128, 8 * CL], BF16, tag="wk", name="wk_sb")
        wv_sb = ipv.tile([128, 8 * CL], BF16, tag="wv", name="wv_sb")

        # DMAs split in halves so the first projection matmuls start early
        nc.sync.dma_start(out=wq_sb[:], in_=wq[:, :])
        for hf in range(2):
            sl = slice(4 * N * hf, 4 * N * (hf + 1))
            nc.sync.dma_start(out=xq_sb[:, sl], in_=xq[:, sl])
        nc.sync.dma_start(out=wk_sb[:], in_=wk[:, :])
        for hf in range(2):
            sl = slice(4 * N * hf, 4 * N * (hf + 1))
            nc.sync.dma_start(out=xk_sb[:, sl], in_=xk[:, sl])
        nc.sync.dma_start(out=wv_sb[:], in_=wv[:, :])
        for hf in range(2):
            sl = slice(4 * N * hf, 4 * N * (hf + 1))
            nc.sync.dma_start(out=xv_sb[:, sl], in_=xv[:, sl])

        xq3 = xq_sb[:].rearrange("p (cc n) -> p cc n", cc=8)
        xk3 = xk_sb[:].rearrange("p (cc n) -> p cc n", cc=8)
        xv3 = xv_sb[:].rearrange("p (cc n) -> p cc n", cc=8)
        wq3 = wq_sb[:].rearrange("p (cc m) -> p cc m", cc=8)
        wk3 = wk_sb[:].rearrange("p (cc m) -> p cc m", cc=8)
        wv3 = wv_sb[:].rearrange("p (cc m) -> p cc m", cc=8)
        vp4 = vpA[:].rearrange("p (kc h x) -> p kc h x", kc=NKC, x=65)
        nc.vector.memset(vp4[:, :, :, 64:65], 1.0)

        def emit_qk_proj(w3, x3, dst, mt, copy_engine, qcps=(0, 1)):
            # channels [128*mt, 128*mt+128) x all 2048 tokens; a spool tile
            # holds a query-chunk pair in its two PSUM banks
            for qcp in qcps:
                ps = spool.tile([128, QP], F32, tag="st",
                                name=f"pj{mt}_{qcp}")
                for cc in range(8):
                    for i in range(2):
                        nc.tensor.matmul(
                            ps[:, 512 * i:512 * (i + 1)],
                            w3[:, cc, 128 * mt:128 * (mt + 1)],
                            x3[:, cc, 512 * (2 * qcp + i):
                               512 * (2 * qcp + i + 1)],
                            start=(cc == 0), stop=(cc == 7))
                for i in range(2):
                    qc = 2 * qcp + i
                    if copy_engine == "scalar":
                        nc.scalar.copy(dst[mt][:, 512 * qc:512 * (qc + 1)],
                                       ps[:, 512 * i:512 * (i + 1)])
                    else:
                        nc.vector.tensor_copy(
                            dst[mt][:, 512 * qc:512 * (qc + 1)],
                            ps[:, 512 * i:512 * (i + 1)])

        def emit_v_proj():
            # V'[128 keys, 4 heads x 64] per key tile; stationary = xv chunk
            for kt in range(NKC):
                psv = spool.tile([128, QP], F32, tag="st", name=f"v{kt}")
                for cc in range(8):
                    nc.tensor.matmul(
                        psv[:, 0:CL],
                        xv3[:, cc, 128 * kt:128 * (kt + 1)],
                        wv3[:, cc, :],
                        start=(cc == 0), stop=(cc == 7))
                nc.vector.tensor_copy(
                    vp4[:, kt, :, 0:64],
                    psv[:, 0:CL].rearrange("p (h d) -> p h d", d=D))

        def emit_pv_mm(pr, kc):
            h, up, P3 = pr[0], pr[1], pr[2]
            if pr[3] is None:
                pr[3] = apool.tile([128, QP], F32, tag="A", name=f"A{h}_{up}")
            At = pr[3]
            for hf in range(2):
                nc.tensor.matmul(
                    At[0:65, 512 * hf:512 * (hf + 1)],
                    vp4[:, kc, h, :],
                    P3[:, kc, 512 * hf:512 * (hf + 1)],
                    start=(kc == 0), stop=(kc == NKC - 1))

        def emit_norm(pr):
            # den copy and row-broadcast ride the DMA engines (no PE throttle)
            h, up, P3, At = pr
            assert At is not None
            dpool = pools["dpool"]
            den = dpool.tile([1, QP], F32, tag="den", name=f"d{h}_{up}")
            nc.vector.tensor_copy(den[:], At[64:65, :])
            dinv = dpool.tile([1, QP], F32, tag="dinv", name=f"r{h}_{up}")
            nc.vector.reciprocal_approx_fast(dinv[:], den[:])
            dib = dpool.tile([64, QP], F32, tag="dib", name=f"b{h}_{up}")
            nc.gpsimd.partition_broadcast(dib[:], dinv[:])
            nc.vector.tensor_mul(
                aT[h // 2][64 * (h % 2):64 * (h % 2) + 64,
                           QP * up:QP * (up + 1)],
                At[0:64, :], dib[:])

        # ---- pipelined emission ----
        units = [(h, up) for h in range(HL) for up in range(2)]
        def after_v():
            emit_v_proj()
            inp_v.close()
            pools["dpool"] = attn.enter_context(
                tc.tile_pool(name="dpool", bufs=1))

        def after_q():
            emit_qk_proj(wq3, xq3, qT, 1, "vector")

        def after_k():
            emit_qk_proj(wk3, xk3, kT, 1, "vector")

        extra = {0: [after_v], 1: [after_q], 2: [after_k]}

        emit_qk_proj(wq3, xq3, qT, 0, "scalar")
        emit_qk_proj(wk3, xk3, kT, 0, "vector")

        prev = None
        for u, (h, up) in enumerate(units):
            p, r = h // 2, h % 2
            Pt = Ppool.tile([128, NKC * QP], BF16, tag="P", name=f"P{u}")
            P3 = Pt[:].rearrange("p (kc q) -> p kc q", kc=NKC)
            for kc in range(NKC):
                st = spool.tile([128, QP], F32, tag="st", name=f"s{u}_{kc}")
                for hf in range(2):
                    nc.tensor.matmul(
                        st[:, 512 * hf:512 * (hf + 1)],
                        kT[p][64 * r:64 * (r + 1), 128 * kc:128 * (kc + 1)],
                        qT[p][64 * r:64 * (r + 1),
                              QP * up + 512 * hf:QP * up + 512 * (hf + 1)],
                        start=True, stop=True)
                if kc < SCALAR_KC:
                    nc.scalar.activation(P3[:, kc, :], st[:], Exp,
                                         scale=float(SCALE))
                else:
                    nc.vector.tensor_scalar(
                        P3[:, kc, :].bitcast(I16), st[:], A16, B16, Mult, Add)
                if prev is not None:
                    emit_pv_mm(prev, kc)
            if prev is not None:
                emit_norm(prev)
            for fn in extra.get(u, []):
                fn()
            prev = [h, up, P3, None]
        for kc in range(NKC):
            emit_pv_mm(prev, kc)
        emit_norm(prev)
        attn.close()

        # ---------------- output projection ----------------
        with ExitStack() as ph:
            opool = ph.enter_context(
                tc.tile_pool(name="opool", bufs=2, space="PSUM"))
            epool = ph.enter_context(tc.tile_pool(name="epool", bufs=3))
            wo_sb = epool.tile([128, 2 * C], BF16, tag="wo", name="wo_sb")
            nc.sync.dma_start(out=wo_sb[:], in_=wo[:, :])
            # qp-major: the qp=0 pieces depend only on norms finished a
            # unit earlier, so the PE isn't queue-blocked on the last
            # unit's normalize chain
            for qp in range(2):
                for mt in range(8):
                    ps = opool.tile([128, QP], F32, tag="po",
                                    name=f"o{mt}_{qp}")
                    for p in range(2):
                        for hf in range(2):
                            nc.tensor.matmul(
                                ps[:, 512 * hf:512 * (hf + 1)],
                                wo_sb[:, C * p + 128 * mt:
                                      C * p + 128 * (mt + 1)],
                                aT[p][:, QP * qp + 512 * hf:
                                      QP * qp + 512 * (hf + 1)],
                                start=(p == 0), stop=(p == 1))
                    ev = epool.tile([128, QP], BF16, tag="ev",
                                    name=f"e{mt}_{qp}")
                    if mt % 2 == 0:
                        nc.scalar.copy(ev[:], ps[:])
                    else:
                        nc.vector.tensor_copy(ev[:], ps[:])
                    nc.sync.dma_start(
                        out=outT[128 * mt:128 * (mt + 1),
                                 QP * qp:QP * (qp + 1)],
                        in_=ev[:])

    nc.compile()
    return nc


def _get_nc():
    if "nc" not in _CACHE:
        _CACHE["nc"] = build_nc()
    return _CACHE["nc"]


def _pack_x(xT, bf):
    # [C, N] -> [128, (cc, N)]
    return np.ascontiguousarray(
        xT.reshape(8, 128, N).transpose(1, 0, 2).reshape(128, 8 * N)).astype(bf)


def _pack_w(w_loc, bf):
    # [C, CL] -> [128, (cc, CL)]
    return np.ascontiguousarray(
        w_loc.reshape(8, 128, CL).transpose(1, 0, 2).reshape(128, 8 * CL)
    ).astype(bf)


def _make_in_maps(q, k, v, Wq, Wk, Wv, Wo):
    bf = ml_dtypes.bfloat16
    q, k, v = np.asarray(q), np.asarray(k), np.asarray(v)
    Wq, Wk, Wv, Wo = (np.asarray(a) for a in (Wq, Wk, Wv, Wo))
    xq_b = [_pack_x(q[b].T, bf) for b in range(B)]
    xk_b = [_pack_x(k[b].T, bf) for b in range(B)]
    xv_b = [_pack_x(v[b].T, bf) for b in range(B)]
    in_maps = []
    for c in range(8):
        b, hg = c // 4, c % 4
        hsl = slice(CL * hg, CL * (hg + 1))
        wo_loc = np.ascontiguousarray(
            Wo[hsl, :].reshape(2, 128, C).transpose(1, 0, 2).reshape(128, 2 * C)
        ).astype(bf)
        in_maps.append({
            "xq": xq_b[b], "xk": xk_b[b], "xv": xv_b[b],
            "wq": _pack_w(Wq[:, hsl], bf),
            "wk": _pack_w(Wk[:, hsl], bf),
            "wv": _pack_w(Wv[:, hsl], bf),
            "wo": wo_loc,
        })
    return in_maps


def _run(inputs, trace=False, **kw):
    nc = _get_nc()
    in_maps = _make_in_maps(inputs["q"], inputs["k"], inputs["v"],
                            inputs["Wq"], inputs["Wk"], inputs["Wv"],
                            inputs["Wo"])
    res = None
    for attempt in range(3):
        try:
            res = run_bass_kernel_spmd(nc, in_maps, core_ids=list(range(8)),
                                       trace=trace, **kw)
            break
        except Exception:
            if attempt == 2:
                raise
            import time
            time.sleep(2.0)
    out = np.zeros((B, N, C), np.float32)
    for c in range(8):
        b = c // 4
        out[b] += res.results[c]["outT"].T.astype(np.float32)
    return out, res


def kernel(**inputs) -> np.ndarray:
    out, _ = _run(inputs, trace=False)
    return out
